# revision 3
# baseline (speedup 1.0000x reference)
"""Bass/Tile TRN2 kernel for nn_InverseSpectralProjection.

Reference: symmetric flip-extension [B,C,H,W] -> [B,C,2H,2W], complex
ifft2 over the last two axes, real part, crop back to [H,W].

The extension makes the signal half-sample symmetric in both axes, so the
ifft2 collapses to a separable cosine transform:

    out[n,m] = mask[n,m] * sum_{h,w} z[h,w] cos(pi n (h+1/2)/H) cos(pi m (w+1/2)/W)
    mask[n,m] = cos(pi n/(2H) + pi m/(2W)) / (H*W)

i.e. out = mask * (C @ z @ C^T) with C[n,h] = cos(pi n (h+1/2)/H).

On the PE (out = lhsT.T @ rhs), with CT = C^T as the moving operand:

    P1 = matmul(lhsT=z,  rhs=CT)   # = z^T  @ CT = (C @ z)^T      [w, n]
    S  = matmul(lhsT=P1, rhs=CT)   # = P1^T @ CT = C @ z @ C^T    [n, m]

so the chain needs no transposes at all.

Sharding: batch dim (8) across the 8 NeuronCores; each core processes 32
independent [256,256] slices (pure data parallelism, no collectives).
"""

import functools
import sys

import numpy as np

for _p in ("/opt/trn_rl_repo",):
    if _p not in sys.path:
        sys.path.append(_p)

B, CCH, H, W = 8, 32, 256, 256
N_CORES = 8
P = 128  # SBUF partitions
KB = H // P  # 2 k-blocks per 256-wide dim


def _constants():
    n = np.arange(H, dtype=np.float64)
    h = np.arange(H, dtype=np.float64)
    # CT[h, n] = cos(pi * n * (h + 1/2) / H)  (= C^T)
    ct = np.cos(np.pi * n[None, :] * (h[:, None] + 0.5) / H).astype(np.float32)
    mask = (
        np.cos(np.pi * n[:, None] / (2 * H) + np.pi * n[None, :] / (2 * W)) / (H * W)
    ).astype(np.float32)
    return np.ascontiguousarray(ct), np.ascontiguousarray(mask)


def build_nc(n_slices: int = CCH):
    import concourse.bass as bass
    import concourse.mybir as mybir
    import concourse.tile as tile
    from concourse import bacc
    from concourse.bass import ts

    fp32 = mybir.dt.float32
    # Bacc (not plain Bass): its compile pipeline moves/splits semaphore
    # waits (move_matmul_waits_to_ldweights, generate_event_semaphores) to
    # satisfy the 1-wait-per-instruction hardware constraint; without it
    # walrus rejects matmuls carrying 2 waits.
    nc = bacc.Bacc(None, debug=False, num_devices=N_CORES)
    z = nc.declare_dram_parameter("z", [n_slices, H, W], fp32, isOutput=False)
    ct = nc.declare_dram_parameter("ct", [H, W], fp32, isOutput=False)
    mask = nc.declare_dram_parameter("mask", [H, W], fp32, isOutput=False)
    out = nc.declare_dram_parameter("out", [n_slices, H, W], fp32, isOutput=True)

    with tile.TileContext(nc) as tc:
        with (
            tc.tile_pool(name="const", bufs=1) as cpool,
            tc.tile_pool(name="io", bufs=4) as iopool,
            tc.tile_pool(name="work", bufs=3) as wpool,
            tc.tile_pool(name="psum", bufs=3, space=bass.MemorySpace.PSUM) as ppool,
        ):
            ct_sb = cpool.tile([P, KB, W], fp32)
            nc.sync.dma_start(ct_sb[:], ct.rearrange("(kb p) n -> p kb n", p=P))
            mask_sb = cpool.tile([P, KB, W], fp32)
            nc.sync.dma_start(mask_sb[:], mask.rearrange("(nb p) m -> p nb m", p=P))

            for c in range(n_slices):
                z_sb = iopool.tile([P, KB, W], fp32, tag="zin")
                nc.sync.dma_start(z_sb[:], z[c].rearrange("(kb p) w -> p kb w", p=P))

                # P1 = z^T @ CT, block rows mb (= w blocks), contraction over kb (= h)
                psum1 = ppool.tile([P, KB, W], fp32, tag="p1")
                for mb in range(KB):
                    for kb in range(KB):
                        nc.tensor.matmul(
                            psum1[:, mb, :],
                            z_sb[:, kb, ts(mb, P)],
                            ct_sb[:, kb, :],
                            start=(kb == 0),
                            stop=(kb == KB - 1),
                        )
                p1_sb = wpool.tile([P, KB, W], fp32, tag="p1sb")
                nc.scalar.copy(p1_sb[:], psum1[:])

                # S = P1^T @ CT, block rows nb (= n blocks), contraction over wb (= w)
                psum2 = ppool.tile([P, KB, W], fp32, tag="p2")
                for nb in range(KB):
                    for wb in range(KB):
                        nc.tensor.matmul(
                            psum2[:, nb, :],
                            p1_sb[:, wb, ts(nb, P)],
                            ct_sb[:, wb, :],
                            start=(wb == 0),
                            stop=(wb == KB - 1),
                        )
                o_sb = iopool.tile([P, KB, W], fp32, tag="zout")
                nc.vector.tensor_mul(o_sb[:], psum2[:], mask_sb[:])
                nc.sync.dma_start(out[c].rearrange("(nb p) m -> p nb m", p=P), o_sb[:])
    nc.compile()
    return nc


@functools.lru_cache(maxsize=1)
def _cached_nc():
    return build_nc(CCH)


def run_on_cores(zeta: np.ndarray, trace: bool = False):
    from concourse.bass_utils import run_bass_kernel_spmd

    ct, mask = _constants()
    in_maps = [
        {"z": np.ascontiguousarray(zeta[i]), "ct": ct, "mask": mask}
        for i in range(N_CORES)
    ]
    res = run_bass_kernel_spmd(
        _cached_nc(), in_maps, core_ids=list(range(N_CORES)), trace=trace
    )
    out = np.stack([res.results[i]["out"] for i in range(N_CORES)], axis=0)
    return out, res


def kernel(zeta: np.ndarray) -> np.ndarray:
    zeta = np.ascontiguousarray(np.asarray(zeta, dtype=np.float32))
    assert zeta.shape == (B, CCH, H, W), zeta.shape
    out, _ = run_on_cores(zeta, trace=False)
    return out.astype(np.float32)


# revision 11
# speedup vs baseline: 1.4371x; 1.4371x over previous
"""Bass/Tile TRN2 kernel for nn_InverseSpectralProjection.

Reference: symmetric flip-extension [B,C,H,W] -> [B,C,2H,2W], complex
ifft2 over the last two axes, real part, crop back to [H,W].

The extension makes the signal half-sample symmetric in both axes, so the
ifft2 collapses to a separable cosine transform:

    out[n,m] = mask[n,m] * sum_{h,w} z[h,w] cos(pi n (h+1/2)/H) cos(pi m (w+1/2)/W)
    mask[n,m] = cos(pi n/(2H) + pi m/(2W)) / (H*W)

i.e. out = mask * (C @ z @ C^T) with C[n,h] = cos(pi n (h+1/2)/H).

On the PE (out = lhsT.T @ rhs), with CT = C^T as the moving operand:

    P1 = matmul(lhsT=z,  rhs=CT)   # = z^T  @ CT = (C @ z)^T      [w, n]
    S  = matmul(lhsT=P1, rhs=CT)   # = P1^T @ CT = C @ z @ C^T    [n, m]

so the chain needs no transposes at all.

Sharding: batch dim (8) across the 8 NeuronCores; each core processes 32
independent [256,256] slices (pure data parallelism, no collectives).
"""

import functools
import sys

import numpy as np

for _p in ("/opt/trn_rl_repo",):
    if _p not in sys.path:
        sys.path.append(_p)

B, CCH, H, W = 8, 32, 256, 256
N_CORES = 8
P = 128  # SBUF partitions
KB = H // P  # 2 k-blocks per 256-wide dim


def _constants():
    n = np.arange(H, dtype=np.float64)
    h = np.arange(H, dtype=np.float64)
    # CT[h, n] = cos(pi * n * (h + 1/2) / H)  (= C^T)
    ct = np.cos(np.pi * n[None, :] * (h[:, None] + 0.5) / H).astype(np.float32)
    mask = (
        np.cos(np.pi * n[:, None] / (2 * H) + np.pi * n[None, :] / (2 * W)) / (H * W)
    ).astype(np.float32)
    return np.ascontiguousarray(ct), np.ascontiguousarray(mask)


def build_nc(n_slices: int = CCH):
    import concourse.bass as bass
    import concourse.mybir as mybir
    import concourse.tile as tile
    from concourse import bacc
    from concourse.bass import ts

    fp32 = mybir.dt.float32
    f32r = mybir.dt.float32r
    # Bacc (not plain Bass): its compile pipeline moves/splits semaphore
    # waits (move_matmul_waits_to_ldweights, generate_event_semaphores) to
    # satisfy the 1-wait-per-instruction hardware constraint; without it
    # walrus rejects matmuls carrying 2 waits.
    nc = bacc.Bacc(None, debug=False, num_devices=N_CORES)
    # z/ct are typed float32r end-to-end (same bits as fp32; numpy binding is
    # float32 either way) so the BIR verifier sees f32r producers feeding the
    # f32r matmuls.
    z = nc.declare_dram_parameter("z", [n_slices, H, W], f32r, isOutput=False)
    ct = nc.declare_dram_parameter("ct", [H, W], f32r, isOutput=False)
    mask = nc.declare_dram_parameter("mask", [H, W], fp32, isOutput=False)
    out = nc.declare_dram_parameter("out", [n_slices, H, W], fp32, isOutput=True)

    with tile.TileContext(nc) as tc:
        with (
            tc.tile_pool(name="const", bufs=1) as cpool,
            tc.tile_pool(name="io", bufs=4) as iopool,
            tc.tile_pool(name="work", bufs=3) as wpool,
            tc.tile_pool(name="psum", bufs=3, space=bass.MemorySpace.PSUM) as ppool,
        ):
            ct_sb = cpool.tile([P, KB, W], f32r)
            nc.sync.dma_start(ct_sb[:], ct.rearrange("(kb p) n -> p kb n", p=P))
            mask_sb = cpool.tile([P, KB, W], fp32)
            nc.sync.dma_start(mask_sb[:], mask.rearrange("(nb p) m -> p nb m", p=P))

            for c in range(n_slices):
                # f32r LDWEIGHTS requires the stationary AP to start 1KB-aligned,
                # so each 128x128 lhsT block lives in its own 256-element slot.
                z_sb = iopool.tile([P, KB, KB, W], f32r, tag="zin")
                for kb in range(KB):
                    nc.sync.dma_start(
                        z_sb[:, kb, :, 0:P],
                        z[c, ts(kb, P), :].rearrange("p (mb w) -> p mb w", w=P),
                    )

                # P1 = z^T @ CT, block rows mb (= w blocks), contraction over kb (= h)
                # float32r: fp32-width operands the PE streams at 1 cycle/row
                # for moving dim >= 256 (vs 4 for plain fp32).
                psum1 = ppool.tile([P, KB, W], fp32, tag="p1")
                for mb in range(KB):
                    for kb in range(KB):
                        nc.tensor.matmul(
                            psum1[:, mb, :],
                            z_sb[:, kb, mb, 0:P],
                            ct_sb[:, kb, :],
                            start=(kb == 0),
                            stop=(kb == KB - 1),
                        )
                p1_sb = wpool.tile([P, KB, KB, W], f32r, tag="p1sb")
                nc.scalar.copy(
                    p1_sb[:, :, :, 0:P],
                    psum1[:].rearrange("p wb (nb w) -> p wb nb w", w=P),
                )

                # S = P1^T @ CT, block rows nb (= n blocks), contraction over wb (= w)
                psum2 = ppool.tile([P, KB, W], fp32, tag="p2")
                for nb in range(KB):
                    for wb in range(KB):
                        nc.tensor.matmul(
                            psum2[:, nb, :],
                            p1_sb[:, wb, nb, 0:P],
                            ct_sb[:, wb, :],
                            start=(wb == 0),
                            stop=(wb == KB - 1),
                        )
                o_sb = iopool.tile([P, KB, W], fp32, tag="zout")
                nc.vector.tensor_mul(o_sb[:], psum2[:], mask_sb[:])
                nc.sync.dma_start(out[c].rearrange("(nb p) m -> p nb m", p=P), o_sb[:])
    nc.compile()
    return nc


@functools.lru_cache(maxsize=1)
def _cached_nc():
    return build_nc(CCH)


def run_on_cores(zeta: np.ndarray, trace: bool = False):
    from concourse.bass_utils import run_bass_kernel_spmd

    ct, mask = _constants()
    in_maps = [
        {"z": np.ascontiguousarray(zeta[i]), "ct": ct, "mask": mask}
        for i in range(N_CORES)
    ]
    res = run_bass_kernel_spmd(
        _cached_nc(), in_maps, core_ids=list(range(N_CORES)), trace=trace
    )
    out = np.stack([res.results[i]["out"] for i in range(N_CORES)], axis=0)
    return out, res


def kernel(zeta: np.ndarray) -> np.ndarray:
    zeta = np.ascontiguousarray(np.asarray(zeta, dtype=np.float32))
    assert zeta.shape == (B, CCH, H, W), zeta.shape
    out, _ = run_on_cores(zeta, trace=False)
    return out.astype(np.float32)
